# revision 35
# baseline (speedup 1.0000x reference)
"""Trainium2 Bass kernel for nn_ClusterMemory_47923245088802.

Computes: loss = mean_b( logsumexp_n(<x_b/||x_b||, f_n>/temp) - <x_b/||x_b||, f_{t_b}>/temp )
with x [4096,1024], f [32768,1024] (rows ~unit norm), t = corrected_targets.

Sharding: features rows split across 8 cores (4096 each, tensor parallel over
num_samples). Each core computes its [4096 x 4096] logit block on the PE array
in fp8-e4m3 DoubleRow mode (f is pre-scaled by 64 on the host to clear the
e4m3 subnormal band), applies exp fused with a row-sum on the scalar engine.

v2 (from trace analysis of the 267us baseline): the main MM stream runs at the
N=512 streaming floor (215.6 ns/MM, 220.8us total), so all recoverable time
was in the 39us startup (13.6us junk warmup + 12.9us of norm/tdot Grams on the
PE) and the 6.8us tail. This version takes everything but the logit matmuls
off the PE:
  - row norms: DVE tensor_mul + reduce_sum over a row-major fp8 copy of x,
    then 1/(temp*64*||x||) on the scalar engine as exp(-0.5*ln(n2)-ln(64*T));
    Ln and Exp live in the same activation table set
    (natural_log_exp_and_others), so zero table thrash. The Ln/Exp chunks are
    interleaved into the ACT queue so they never delay a main exp.
  - target dots: DVE tensor_mul + reduce_sum of x-slice * gathered-f rows
    (fp8, 64-scaled; the 64s cancel against the norm scale exactly).
  - DMA layouts are contiguous per partition (4KB+ descriptors) and sliced so
    the first matmul's inputs (f cols 0-511 + x rows 0-511) land first.
  - tail: the last group's exp is split into 4x512 activations that pipeline
    with its matmuls; sumexp is DMA'd out in chunks as tiles finish.
The host combines the 8 partial sum-exps with a log (the cross-shard
all-reduce of the CE log-sum-exp) and takes the mean.
"""

import math

import numpy as np
import ml_dtypes

B = 4096          # batch
D = 1024          # feature dim (contraction)
NTOT = 32768      # num_samples
TEMP = 0.05
NCORES = 8
NS = NTOT // NCORES   # samples per core
P = 128
KO = D // P           # 8 k-chunks
BT = B // P           # 32 batch tiles
TT = BT // NCORES     # 4 batch tiles per core for the target-dot shard
NJ = NS // 512        # 8 n-slices of 512
FSCALE = 64.0         # host pre-scale on f before e4m3 quantization

_CACHE = {}


def _build_nc():
    from contextlib import ExitStack

    import concourse.bass as bass
    import concourse.bacc as bacc
    import concourse.mybir as mybir
    import concourse.tile as tile

    f32 = mybir.dt.float32
    fp8 = mybir.dt.float8e4
    AF = mybir.ActivationFunctionType
    DR = mybir.MatmulPerfMode.DoubleRow
    ts = bass.ts

    nc = bacc.Bacc("TRN2", target_bir_lowering=False, debug=False,
                   enable_asserts=False)

    # DRAM layouts are contiguous per partition (see _prep_in_maps).
    x8 = nc.dram_tensor("x8", [KO, P, KO, 512], fp8, kind="ExternalInput")
    f8 = nc.dram_tensor("f8", [NJ, P, KO, 512], fp8, kind="ExternalInput")
    xbd = nc.dram_tensor("xbd", [P, BT, D], fp8, kind="ExternalInput")
    xsl = nc.dram_tensor("xsl", [P, TT, D], fp8, kind="ExternalInput")
    fsel = nc.dram_tensor("fsel", [P, TT, D], fp8, kind="ExternalInput")
    sumexp_out = nc.dram_tensor("sumexp", [P, BT], f32, kind="ExternalOutput")
    tdot_out = nc.dram_tensor("tdot", [P, TT], f32, kind="ExternalOutput")
    s8_out = nc.dram_tensor("s8", [P, BT], f32, kind="ExternalOutput")

    with tile.TileContext(nc) as tc, ExitStack() as ctx:
        consts = ctx.enter_context(tc.tile_pool(name="consts", bufs=1))
        big = ctx.enter_context(tc.tile_pool(name="big", bufs=1))
        stats = ctx.enter_context(tc.tile_pool(name="stats", bufs=1))

        # Slice-major SBUF layouts: each 512-column slice is a contiguous
        # 4KB block per partition, so its DMA uses 4KB descriptors. (A
        # [P, KO, B] layout scatters each slice into 512B runs, and the
        # SDMA packet round-robin then starves these transfers 8:1 against
        # the gpsimd queue's 4KB packets.)
        x_sb = big.tile([P, KO, KO, 512], fp8)
        f_sb = big.tile([P, NJ, KO, 512], fp8)
        xbd_sb = big.tile([P, BT, D], fp8)
        xsl_sb = big.tile([P, TT, D], fp8)
        fsel_sb = big.tile([P, TT, D], fp8)

        bf16 = mybir.dt.bfloat16
        wz = consts.tile([P, 512], fp8)        # junk warmup operand
        dummy = consts.tile([P, 2048], f32)    # unused act main output
        scratch = consts.tile([P, 4, D], bf16)  # elementwise-product scratch
        lnb = consts.tile([P, 1], f32)         # -ln(64*temp) bias for Exp
        n2p = stats.tile([P, BT], f32)         # sum_d x^2 per batch row
        lnn = stats.tile([P, BT], f32)
        s8 = stats.tile([P, BT], f32)          # 1/(temp*64*norm) -> s8_out
        sacc = stats.tile([P, BT, 4], f32)     # per-(tile,quarter) exp accums
        tail5 = stats.tile([P, 5], f32)        # tile 31: 3 accums + 2 splits
        sumexp_sb = stats.tile([P, BT], f32)
        tdot_sb = stats.tile([P, TT], f32)

        # ---- input DMAs. All x8/f8 go on the sync HWDGE ring in strict
        # first-use order - ring FIFO makes emission order a priority order,
        # so the first matmul's inputs (f8 slice 0, x8 slice 0) get the HBM
        # bandwidth first. The scalar ring carries NO DMAs: issue
        # instructions there would block the ACT queue between exps.
        # gpsimd (SWDGE, separate path): the row-major x copy for norms +
        # the tdot operands, also in need-order.
        nc.scalar.dma_start(x_sb[:, 0], x8.ap()[0])
        nc.scalar.dma_start(xbd_sb[:, 0:2, :], xbd.ap()[:, 0:2, :])
        nc.scalar.dma_start(xbd_sb[:, 2:4, :], xbd.ap()[:, 2:4, :])
        nc.sync.dma_start(f_sb[:, 0], f8.ap()[0])
        nc.sync.dma_start(f_sb[:, 1], f8.ap()[1])
        nc.sync.dma_start(f_sb[:, 2], f8.ap()[2])
        nc.sync.dma_start(xbd_sb[:, 4:8, :], xbd.ap()[:, 4:8, :])
        nc.sync.dma_start(f_sb[:, 3], f8.ap()[3])
        nc.sync.dma_start(x_sb[:, 1], x8.ap()[1])
        nc.sync.dma_start(f_sb[:, 4], f8.ap()[4])
        nc.sync.dma_start(f_sb[:, 5], f8.ap()[5])
        nc.sync.dma_start(xbd_sb[:, 8:16, :], xbd.ap()[:, 8:16, :])
        nc.sync.dma_start(f_sb[:, 6], f8.ap()[6])
        nc.sync.dma_start(f_sb[:, 7], f8.ap()[7])
        nc.sync.dma_start(x_sb[:, 2], x8.ap()[2])
        nc.sync.dma_start(x_sb[:, 3], x8.ap()[3])
        nc.sync.dma_start(xbd_sb[:, 16:32, :], xbd.ap()[:, 16:32, :])
        for j in range(4, KO):
            nc.sync.dma_start(x_sb[:, j], x8.ap()[j])
        nc.sync.dma_start(xsl_sb[:], xsl.ap())
        nc.sync.dma_start(fsel_sb[:], fsel.ap())
        # The rest of the row-major x copy goes on the SWDGE path, gated
        # (via a WAW dep on a one-element DVE memset placed after the first
        # norm reduce) so its large packets can't starve the f8/x8 stream
        # during the critical first ~15us. The gpsimd queue serializes the
        # transfers behind the gated one.


        # ---- HAM warmup: junk matmuls over a zeroed tile cover the first
        # input DMA window and release the PE clock gate (1.2 -> 2.4 GHz).
        nc.vector.memset(wz[:], 0.0)
        nc.vector.memset(lnb[:], -math.log(TEMP * FSCALE))
        with tc.tile_pool(name="warm", bufs=2, space="PSUM") as warm:
            for w in range(8):
                pw = warm.tile([P, 512], f32)
                nc.tensor.matmul(pw[:], wz[:, :P], wz[:], start=True,
                                 stop=True)

        # ---- norms (DVE) + scale (ACT, Ln/Exp in one table set) ----
        def norm_chunk(a, b):
            w = b - a
            nc.vector.tensor_mul(scratch[:, :w, :], xbd_sb[:, a:b, :],
                                 xbd_sb[:, a:b, :])
            nc.vector.reduce_sum(n2p[:, a:b], scratch[:, :w, :],
                                 axis=mybir.AxisListType.X)

        def scale_chunk(a, b):
            nc.scalar.activation(lnn[:, a:b], n2p[:, a:b], AF.Ln)
            nc.scalar.activation(s8[:, a:b], lnn[:, a:b], AF.Exp,
                                 bias=lnb[:], scale=-0.5)

        norm_chunk(0, 1)
        scale_chunk(0, 1)
        norm_chunk(1, 2)
        scale_chunk(1, 2)
        norm_chunk(2, 4)
        scale_chunk(2, 4)
        norm_chunk(4, 5)
        norm_chunk(5, 6)
        norm_chunk(6, 8)
        for t in range(8, 32, 4):
            norm_chunk(t, t + 4)

        # ---- target dots (DVE): tdot_raw = sum_d x8*fsel = 64*<x,f_t> ----
        nc.vector.tensor_mul(scratch[:], xsl_sb[:], fsel_sb[:])
        nc.vector.reduce_sum(tdot_sb[:], scratch[:],
                             axis=mybir.AxisListType.X)
        nc.sync.dma_start(tdot_out.ap(), tdot_sb[:])

        # ---- main: [4096 x 4096] logits in fp8 DoubleRow, exp + row-sum.
        # 2 slices share one 2-bank psum tile; 4 pool buffers let the PE
        # run up to 3 groups ahead of the ACT stream, so the interleaved
        # Ln/Exp scale chunks never stall the matmul pipeline.
        NG = 2
        with tc.tile_pool(name="psm", bufs=4, space="PSUM") as psm:

            def emit_group(i, jj, split=False):
                pl = psm.tile([P, NG * 512], f32)
                for g in range(NG):
                    j = jj * NG + g
                    for k2 in range(KO // 2):
                        nc.tensor.matmul(
                            pl[:, g * 512:(g + 1) * 512],
                            x_sb[:, i // 4, 2 * k2:2 * k2 + 2, ts(i % 4, P)],
                            f_sb[:, j, 2 * k2:2 * k2 + 2, :],
                            start=k2 == 0, stop=k2 == KO // 2 - 1,
                            perf_mode=DR)
                if i == BT - 1:
                    # last tile: separate accum slots; the very last group's
                    # exps are split 512-wide so they pipeline with its
                    # matmuls, shortening the end-of-kernel drain.
                    if split:
                        for g in range(NG):
                            nc.scalar.activation(
                                dummy[:, g * 512:(g + 1) * 512],
                                pl[:, g * 512:(g + 1) * 512], AF.Exp,
                                bias=0.0, scale=s8[:, i:i + 1],
                                accum_out=tail5[:, 3 + g:4 + g])
                    else:
                        nc.scalar.activation(dummy[:, :1024], pl[:], AF.Exp,
                                             bias=0.0, scale=s8[:, i:i + 1],
                                             accum_out=tail5[:, jj:jj + 1])
                else:
                    nc.scalar.activation(dummy[:, :1024], pl[:], AF.Exp,
                                         bias=0.0, scale=s8[:, i:i + 1],
                                         accum_out=sacc[:, i, jj:jj + 1])

            def finish(a, b):
                nc.vector.reduce_sum(sumexp_sb[:, a:b], sacc[:, a:b, :],
                                     axis=mybir.AxisListType.X)
                nc.sync.dma_start(sumexp_out.ap()[:, a:b], sumexp_sb[:, a:b])

            # First 8 tiles, slice-pair-major: each 4-group pass reuses one
            # pair of f slices (or one x slice), stretching ~0.5MB of fresh
            # DMA over ~7us of matmuls so the input stream keeps up.
            for i in range(4):
                emit_group(i, 0)           # f slices 0,1 | x slice 0
            for i in range(4):
                emit_group(i, 1)           # f slices 2,3
                if i == 1:
                    scale_chunk(4, 5)
                elif i == 2:
                    scale_chunk(5, 6)
            for i in range(4, 8):
                emit_group(i, 0)           # x slice 1
                if i == 5:
                    scale_chunk(6, 8)
            for i in range(4, 8):
                emit_group(i, 1)
                if i == 5:
                    scale_chunk(8, 16)
            for i in range(8):
                emit_group(i, 2)           # f slices 4,5
                if i == 5:
                    scale_chunk(16, 24)
            for i in range(8):
                emit_group(i, 3)           # f slices 6,7
                if i == 3:
                    scale_chunk(24, 32)
                    nc.sync.dma_start(s8_out.ap(), s8[:])
                elif i == 7:
                    finish(0, 8)
            for i in range(8, BT):
                for jj in range(4):
                    emit_group(i, jj, split=(i == BT - 1 and jj == 3))
                if i == 15:
                    finish(8, 16)
                elif i == 23:
                    finish(16, 24)
                elif i == 30:
                    finish(24, 31)
            nc.vector.reduce_sum(sumexp_sb[:, BT - 1:BT], tail5[:],
                                 axis=mybir.AxisListType.X)
            nc.sync.dma_start(sumexp_out.ap()[:, BT - 1:BT],
                              sumexp_sb[:, BT - 1:BT])

    # The act-table insertion pass picks a table set per activation function
    # independently, so Exp lands in exp_and_others and Ln in natural_log -
    # and every interleaved Ln/Exp scale chunk then costs two ~1.3us
    # ACT_TABLE_LOADs in the middle of the exp stream. Both functions are
    # served by the natural_log_exp_and_others set, so for this compile we
    # present the pass a view of the tables (same entries, same order, so
    # the file-indexed set ids stay valid) where that combined set is the
    # only one offering Exp/Ln. One table load at kernel entry, no thrash.
    import concourse.bacc as bacc_mod
    from concourse.hw_specs import get_activation_tables
    tabs = get_activation_tables(nc.m.arch)
    patched = {
        name: (funcs if name == "natural_log_exp_and_others"
               else funcs - {AF.Exp, AF.Ln})
        for name, funcs in tabs.items()
    }
    orig_fn = bacc_mod.get_activation_tables
    bacc_mod.get_activation_tables = (
        lambda arch: patched if arch == nc.m.arch else orig_fn(arch))
    try:
        nc.compile()
    finally:
        bacc_mod.get_activation_tables = orig_fn
    return nc


def _get_nc():
    if "nc" not in _CACHE:
        _CACHE["nc"] = _build_nc()
    return _CACHE["nc"]


def _prep_in_maps(inputs, corrected_targets, features):
    import concourse.mybir as mybir
    fp8 = mybir.dt.np(mybir.dt.float8e4)
    x = np.asarray(inputs, dtype=np.float32)
    f = np.asarray(features, dtype=np.float32)
    ct = np.asarray(corrected_targets).astype(np.int64)

    x8q = x.astype(fp8)                                          # [B, D]
    # x8d[j, p, ko, b] = x^[j*512+b, ko*128+p]; per-partition contiguous 4KB
    x8d = np.ascontiguousarray(
        x8q.reshape(KO, 512, KO, P).transpose(0, 3, 2, 1))
    # xbd[p, t, d] = x^[t*128+p, d]
    xbd = np.ascontiguousarray(x8q.reshape(BT, P, D).transpose(1, 0, 2))
    fs_all = (f[ct] * FSCALE).astype(fp8)                        # [B, D]

    in_maps = []
    for c in range(NCORES):
        fc = (f[c * NS:(c + 1) * NS] * FSCALE).astype(fp8)       # [NS, D]
        f8d = np.ascontiguousarray(
            fc.reshape(NJ, 512, KO, P).transpose(0, 3, 2, 1))
        fsel = np.ascontiguousarray(
            fs_all[c * 512:(c + 1) * 512].reshape(TT, P, D).transpose(1, 0, 2))
        xsl = np.ascontiguousarray(xbd[:, c * TT:(c + 1) * TT, :])
        in_maps.append({
            "x8": x8d, "f8": f8d, "xbd": xbd, "xsl": xsl, "fsel": fsel,
        })
    return in_maps


def _combine(results):
    S = np.zeros(B, dtype=np.float64)
    for c in range(NCORES):
        S += results[c]["sumexp"].astype(np.float64).T.ravel()
    s8 = results[0]["s8"].astype(np.float64).T.ravel()
    tdot_raw = np.concatenate(
        [results[c]["tdot"].astype(np.float64).T.ravel() for c in range(NCORES)])
    lse = np.log(S)
    loss = np.mean(lse - tdot_raw * s8)
    return np.asarray(loss, dtype=np.float32)


def _run(inputs, targets, corrected_targets, features, trace=False, tmpdir=None):
    import time
    from concourse import bass_utils
    nc = _get_nc()
    in_maps = _prep_in_maps(inputs, corrected_targets, features)
    last_exc = None
    for attempt in range(3):
        try:
            res = bass_utils.run_bass_kernel_spmd(
                nc, in_maps, core_ids=list(range(NCORES)), trace=trace,
                tmpdir=tmpdir)
            return _combine(res.results), res
        except Exception as e:  # transient device state (e.g. prior crash)
            last_exc = e
            time.sleep(2.0)
    raise last_exc


def kernel(inputs, targets, corrected_targets, features):
    out, _ = _run(inputs, targets, corrected_targets, features, trace=False)
    return out


# revision 38
# speedup vs baseline: 1.0007x; 1.0007x over previous
"""Trainium2 Bass kernel for nn_ClusterMemory_47923245088802.

Computes: loss = mean_b( logsumexp_n(<x_b/||x_b||, f_n>/temp) - <x_b/||x_b||, f_{t_b}>/temp )
with x [4096,1024], f [32768,1024] (rows ~unit norm), t = corrected_targets.

Sharding: features rows split across 8 cores (4096 each, tensor parallel over
num_samples). Each core computes its [4096 x 4096] logit block on the PE array
in fp8-e4m3 DoubleRow mode (f is pre-scaled by 64 on the host to clear the
e4m3 subnormal band), applies exp fused with a row-sum on the scalar engine.

v2 (from trace analysis of the 267us baseline): the main MM stream runs at the
N=512 streaming floor (215.6 ns/MM, 220.8us total), so all recoverable time
was in the 39us startup (13.6us junk warmup + 12.9us of norm/tdot Grams on the
PE) and the 6.8us tail. This version takes everything but the logit matmuls
off the PE:
  - row norms: DVE tensor_mul + reduce_sum over a row-major fp8 copy of x,
    then 1/(temp*64*||x||) on the scalar engine as exp(-0.5*ln(n2)-ln(64*T));
    Ln and Exp live in the same activation table set
    (natural_log_exp_and_others), so zero table thrash. The Ln/Exp chunks are
    interleaved into the ACT queue so they never delay a main exp.
  - target dots: DVE tensor_mul + reduce_sum of x-slice * gathered-f rows
    (fp8, 64-scaled; the 64s cancel against the norm scale exactly).
  - DMA layouts are contiguous per partition (4KB+ descriptors) and sliced so
    the first matmul's inputs (f cols 0-511 + x rows 0-511) land first.
  - tail: the last group's exp is split into 4x512 activations that pipeline
    with its matmuls; sumexp is DMA'd out in chunks as tiles finish.
The host combines the 8 partial sum-exps with a log (the cross-shard
all-reduce of the CE log-sum-exp) and takes the mean.
"""

import math

import numpy as np
import ml_dtypes

B = 4096          # batch
D = 1024          # feature dim (contraction)
NTOT = 32768      # num_samples
TEMP = 0.05
NCORES = 8
NS = NTOT // NCORES   # samples per core
P = 128
KO = D // P           # 8 k-chunks
BT = B // P           # 32 batch tiles
TT = BT // NCORES     # 4 batch tiles per core for the target-dot shard
NJ = NS // 512        # 8 n-slices of 512
FSCALE = 64.0         # host pre-scale on f before e4m3 quantization

_CACHE = {}


def _build_nc():
    from contextlib import ExitStack

    import concourse.bass as bass
    import concourse.bacc as bacc
    import concourse.mybir as mybir
    import concourse.tile as tile

    f32 = mybir.dt.float32
    fp8 = mybir.dt.float8e4
    AF = mybir.ActivationFunctionType
    DR = mybir.MatmulPerfMode.DoubleRow
    ts = bass.ts

    nc = bacc.Bacc("TRN2", target_bir_lowering=False, debug=False,
                   enable_asserts=False)

    # DRAM layouts are contiguous per partition (see _prep_in_maps).
    x8 = nc.dram_tensor("x8", [KO, P, KO, 512], fp8, kind="ExternalInput")
    f8 = nc.dram_tensor("f8", [NJ, P, KO, 512], fp8, kind="ExternalInput")
    xbd = nc.dram_tensor("xbd", [P, BT, D], fp8, kind="ExternalInput")
    xsl = nc.dram_tensor("xsl", [P, TT, D], fp8, kind="ExternalInput")
    fsel = nc.dram_tensor("fsel", [P, TT, D], fp8, kind="ExternalInput")
    sumexp_out = nc.dram_tensor("sumexp", [P, BT], f32, kind="ExternalOutput")
    tdot_out = nc.dram_tensor("tdot", [P, TT], f32, kind="ExternalOutput")
    s8_out = nc.dram_tensor("s8", [P, BT], f32, kind="ExternalOutput")

    with tile.TileContext(nc) as tc, ExitStack() as ctx:
        consts = ctx.enter_context(tc.tile_pool(name="consts", bufs=1))
        big = ctx.enter_context(tc.tile_pool(name="big", bufs=1))
        stats = ctx.enter_context(tc.tile_pool(name="stats", bufs=1))

        # Slice-major SBUF layouts: each 512-column slice is a contiguous
        # 4KB block per partition, so its DMA uses 4KB descriptors. (A
        # [P, KO, B] layout scatters each slice into 512B runs, and the
        # SDMA packet round-robin then starves these transfers 8:1 against
        # the gpsimd queue's 4KB packets.)
        x_sb = big.tile([P, KO, KO, 512], fp8)
        f_sb = big.tile([P, NJ, KO, 512], fp8)
        xbd_sb = big.tile([P, BT, D], fp8)
        xsl_sb = big.tile([P, TT, D], fp8)
        fsel_sb = big.tile([P, TT, D], fp8)

        bf16 = mybir.dt.bfloat16
        wz = consts.tile([P, 512], fp8)        # junk warmup operand
        dummy = consts.tile([P, 2048], f32)    # unused act main output
        scratch = consts.tile([P, 4, D], bf16)  # elementwise-product scratch
        lnb = consts.tile([P, 1], f32)         # -ln(64*temp) bias for Exp
        n2p = stats.tile([P, BT], f32)         # sum_d x^2 per batch row
        lnn = stats.tile([P, BT], f32)
        s8 = stats.tile([P, BT], f32)          # 1/(temp*64*norm) -> s8_out
        sacc = stats.tile([P, BT, 4], f32)     # per-(tile,quarter) exp accums
        tail5 = stats.tile([P, 5], f32)        # tile 31: 3 accums + 2 splits
        sumexp_sb = stats.tile([P, BT], f32)
        tdot_sb = stats.tile([P, TT], f32)

        # ---- input DMAs. All x8/f8 go on the sync HWDGE ring in strict
        # first-use order - ring FIFO makes emission order a priority order,
        # so the first matmul's inputs (f8 slice 0, x8 slice 0) get the HBM
        # bandwidth first. The scalar ring carries NO DMAs: issue
        # instructions there would block the ACT queue between exps.
        # gpsimd (SWDGE, separate path): the row-major x copy for norms +
        # the tdot operands, also in need-order.
        nc.scalar.dma_start(x_sb[:, 0], x8.ap()[0])
        nc.scalar.dma_start(xbd_sb[:, 0:2, :], xbd.ap()[:, 0:2, :])
        nc.scalar.dma_start(xbd_sb[:, 2:4, :], xbd.ap()[:, 2:4, :])
        nc.sync.dma_start(f_sb[:, 0], f8.ap()[0])
        nc.sync.dma_start(f_sb[:, 1], f8.ap()[1])
        nc.sync.dma_start(f_sb[:, 2], f8.ap()[2])
        nc.sync.dma_start(f_sb[:, 3], f8.ap()[3])
        nc.sync.dma_start(x_sb[:, 1], x8.ap()[1])
        nc.sync.dma_start(xbd_sb[:, 4:8, :], xbd.ap()[:, 4:8, :])
        nc.sync.dma_start(f_sb[:, 4], f8.ap()[4])
        nc.sync.dma_start(f_sb[:, 5], f8.ap()[5])
        nc.sync.dma_start(xbd_sb[:, 8:16, :], xbd.ap()[:, 8:16, :])
        nc.sync.dma_start(f_sb[:, 6], f8.ap()[6])
        nc.sync.dma_start(f_sb[:, 7], f8.ap()[7])
        nc.sync.dma_start(x_sb[:, 2], x8.ap()[2])
        nc.sync.dma_start(x_sb[:, 3], x8.ap()[3])
        nc.sync.dma_start(xbd_sb[:, 16:32, :], xbd.ap()[:, 16:32, :])
        for j in range(4, KO):
            nc.sync.dma_start(x_sb[:, j], x8.ap()[j])
        nc.sync.dma_start(xsl_sb[:], xsl.ap())
        nc.sync.dma_start(fsel_sb[:], fsel.ap())
        # The rest of the row-major x copy goes on the SWDGE path, gated
        # (via a WAW dep on a one-element DVE memset placed after the first
        # norm reduce) so its large packets can't starve the f8/x8 stream
        # during the critical first ~15us. The gpsimd queue serializes the
        # transfers behind the gated one.


        # ---- HAM warmup: junk matmuls over a zeroed tile cover the first
        # input DMA window and release the PE clock gate (1.2 -> 2.4 GHz).
        nc.vector.memset(wz[:], 0.0)
        nc.vector.memset(lnb[:], -math.log(TEMP * FSCALE))
        with tc.tile_pool(name="warm", bufs=2, space="PSUM") as warm:
            for w in range(8):
                pw = warm.tile([P, 512], f32)
                nc.tensor.matmul(pw[:], wz[:, :P], wz[:], start=True,
                                 stop=True)

        # ---- norms (DVE) + scale (ACT, Ln/Exp in one table set) ----
        def norm_chunk(a, b):
            w = b - a
            nc.vector.tensor_mul(scratch[:, :w, :], xbd_sb[:, a:b, :],
                                 xbd_sb[:, a:b, :])
            nc.vector.reduce_sum(n2p[:, a:b], scratch[:, :w, :],
                                 axis=mybir.AxisListType.X)

        def scale_chunk(a, b):
            nc.scalar.activation(lnn[:, a:b], n2p[:, a:b], AF.Ln)
            nc.scalar.activation(s8[:, a:b], lnn[:, a:b], AF.Exp,
                                 bias=lnb[:], scale=-0.5)

        norm_chunk(0, 1)
        scale_chunk(0, 1)
        norm_chunk(1, 2)
        scale_chunk(1, 2)
        norm_chunk(2, 4)
        scale_chunk(2, 4)
        norm_chunk(4, 6)
        norm_chunk(6, 8)
        for t in range(8, 32, 4):
            norm_chunk(t, t + 4)

        # ---- target dots (DVE): tdot_raw = sum_d x8*fsel = 64*<x,f_t> ----
        nc.vector.tensor_mul(scratch[:], xsl_sb[:], fsel_sb[:])
        nc.vector.reduce_sum(tdot_sb[:], scratch[:],
                             axis=mybir.AxisListType.X)
        nc.sync.dma_start(tdot_out.ap(), tdot_sb[:])

        # ---- main: [4096 x 4096] logits in fp8 DoubleRow, exp + row-sum.
        # 2 slices share one 2-bank psum tile; 4 pool buffers let the PE
        # run up to 3 groups ahead of the ACT stream, so the interleaved
        # Ln/Exp scale chunks never stall the matmul pipeline.
        NG = 2
        with tc.tile_pool(name="psm", bufs=4, space="PSUM") as psm:

            def emit_group(i, jj, split=False):
                pl = psm.tile([P, NG * 512], f32)
                for g in range(NG):
                    j = jj * NG + g
                    for k2 in range(KO // 2):
                        nc.tensor.matmul(
                            pl[:, g * 512:(g + 1) * 512],
                            x_sb[:, i // 4, 2 * k2:2 * k2 + 2, ts(i % 4, P)],
                            f_sb[:, j, 2 * k2:2 * k2 + 2, :],
                            start=k2 == 0, stop=k2 == KO // 2 - 1,
                            perf_mode=DR)
                if i == BT - 1:
                    # last tile: separate accum slots; the very last group's
                    # exps are split 512-wide so they pipeline with its
                    # matmuls, shortening the end-of-kernel drain.
                    if split:
                        for g in range(NG):
                            nc.scalar.activation(
                                dummy[:, g * 512:(g + 1) * 512],
                                pl[:, g * 512:(g + 1) * 512], AF.Exp,
                                bias=0.0, scale=s8[:, i:i + 1],
                                accum_out=tail5[:, 3 + g:4 + g])
                    else:
                        nc.scalar.activation(dummy[:, :1024], pl[:], AF.Exp,
                                             bias=0.0, scale=s8[:, i:i + 1],
                                             accum_out=tail5[:, jj:jj + 1])
                else:
                    nc.scalar.activation(dummy[:, :1024], pl[:], AF.Exp,
                                         bias=0.0, scale=s8[:, i:i + 1],
                                         accum_out=sacc[:, i, jj:jj + 1])

            def finish(a, b):
                nc.vector.reduce_sum(sumexp_sb[:, a:b], sacc[:, a:b, :],
                                     axis=mybir.AxisListType.X)
                nc.sync.dma_start(sumexp_out.ap()[:, a:b], sumexp_sb[:, a:b])

            # First 8 tiles, slice-pair-major: each 4-group pass reuses one
            # pair of f slices (or one x slice), stretching ~0.5MB of fresh
            # DMA over ~7us of matmuls so the input stream keeps up.
            for i in range(4):
                emit_group(i, 0)           # f slices 0,1 | x slice 0
            for i in range(4):
                emit_group(i, 1)           # f slices 2,3
                if i == 2:
                    scale_chunk(4, 6)
            for i in range(4, 8):
                emit_group(i, 0)           # x slice 1
                if i == 5:
                    scale_chunk(6, 8)
            for i in range(4, 8):
                emit_group(i, 1)
                if i == 5:
                    scale_chunk(8, 16)
            for i in range(8):
                emit_group(i, 2)           # f slices 4,5
                if i == 5:
                    scale_chunk(16, 24)
            for i in range(8):
                emit_group(i, 3)           # f slices 6,7
                if i == 3:
                    scale_chunk(24, 32)
                    nc.sync.dma_start(s8_out.ap(), s8[:])
                elif i == 7:
                    finish(0, 8)
            for i in range(8, BT):
                for jj in range(4):
                    emit_group(i, jj, split=(i == BT - 1 and jj == 3))
                if i == 15:
                    finish(8, 16)
                elif i == 23:
                    finish(16, 24)
                elif i == 30:
                    finish(24, 31)
            nc.vector.reduce_sum(sumexp_sb[:, BT - 1:BT], tail5[:],
                                 axis=mybir.AxisListType.X)
            nc.sync.dma_start(sumexp_out.ap()[:, BT - 1:BT],
                              sumexp_sb[:, BT - 1:BT])

    # The act-table insertion pass picks a table set per activation function
    # independently, so Exp lands in exp_and_others and Ln in natural_log -
    # and every interleaved Ln/Exp scale chunk then costs two ~1.3us
    # ACT_TABLE_LOADs in the middle of the exp stream. Both functions are
    # served by the natural_log_exp_and_others set, so for this compile we
    # present the pass a view of the tables (same entries, same order, so
    # the file-indexed set ids stay valid) where that combined set is the
    # only one offering Exp/Ln. One table load at kernel entry, no thrash.
    import concourse.bacc as bacc_mod
    from concourse.hw_specs import get_activation_tables
    tabs = get_activation_tables(nc.m.arch)
    patched = {
        name: (funcs if name == "natural_log_exp_and_others"
               else funcs - {AF.Exp, AF.Ln})
        for name, funcs in tabs.items()
    }
    orig_fn = bacc_mod.get_activation_tables
    bacc_mod.get_activation_tables = (
        lambda arch: patched if arch == nc.m.arch else orig_fn(arch))
    try:
        nc.compile()
    finally:
        bacc_mod.get_activation_tables = orig_fn
    return nc


def _get_nc():
    if "nc" not in _CACHE:
        _CACHE["nc"] = _build_nc()
    return _CACHE["nc"]


def _prep_in_maps(inputs, corrected_targets, features):
    import concourse.mybir as mybir
    fp8 = mybir.dt.np(mybir.dt.float8e4)
    x = np.asarray(inputs, dtype=np.float32)
    f = np.asarray(features, dtype=np.float32)
    ct = np.asarray(corrected_targets).astype(np.int64)

    x8q = x.astype(fp8)                                          # [B, D]
    # x8d[j, p, ko, b] = x^[j*512+b, ko*128+p]; per-partition contiguous 4KB
    x8d = np.ascontiguousarray(
        x8q.reshape(KO, 512, KO, P).transpose(0, 3, 2, 1))
    # xbd[p, t, d] = x^[t*128+p, d]
    xbd = np.ascontiguousarray(x8q.reshape(BT, P, D).transpose(1, 0, 2))
    fs_all = (f[ct] * FSCALE).astype(fp8)                        # [B, D]

    in_maps = []
    for c in range(NCORES):
        fc = (f[c * NS:(c + 1) * NS] * FSCALE).astype(fp8)       # [NS, D]
        f8d = np.ascontiguousarray(
            fc.reshape(NJ, 512, KO, P).transpose(0, 3, 2, 1))
        fsel = np.ascontiguousarray(
            fs_all[c * 512:(c + 1) * 512].reshape(TT, P, D).transpose(1, 0, 2))
        xsl = np.ascontiguousarray(xbd[:, c * TT:(c + 1) * TT, :])
        in_maps.append({
            "x8": x8d, "f8": f8d, "xbd": xbd, "xsl": xsl, "fsel": fsel,
        })
    return in_maps


def _combine(results):
    S = np.zeros(B, dtype=np.float64)
    for c in range(NCORES):
        S += results[c]["sumexp"].astype(np.float64).T.ravel()
    s8 = results[0]["s8"].astype(np.float64).T.ravel()
    tdot_raw = np.concatenate(
        [results[c]["tdot"].astype(np.float64).T.ravel() for c in range(NCORES)])
    lse = np.log(S)
    loss = np.mean(lse - tdot_raw * s8)
    return np.asarray(loss, dtype=np.float32)


def _run(inputs, targets, corrected_targets, features, trace=False, tmpdir=None):
    import time
    from concourse import bass_utils
    nc = _get_nc()
    in_maps = _prep_in_maps(inputs, corrected_targets, features)
    last_exc = None
    for attempt in range(3):
        try:
            res = bass_utils.run_bass_kernel_spmd(
                nc, in_maps, core_ids=list(range(NCORES)), trace=trace,
                tmpdir=tmpdir)
            return _combine(res.results), res
        except Exception as e:  # transient device state (e.g. prior crash)
            last_exc = e
            time.sleep(2.0)
    raise last_exc


def kernel(inputs, targets, corrected_targets, features):
    out, _ = _run(inputs, targets, corrected_targets, features, trace=False)
    return out
